# revision 8
# baseline (speedup 1.0000x reference)
"""Trainium2 Bass kernel for AttentionGNNLayer (8-core SPMD).

Strategy: shard destination nodes across 8 cores (12500 each). Host buckets
edges by aggregation target into 128-node windows; device gathers source
embeddings via batched indirect DMA and scatter-adds via one-hot matmuls into
PSUM. Segment softmax is computed without max-subtraction (logits are
tanh-bounded), so numerator and denominator come out of one accumulation pass.
"""

import sys

for _p in ("/opt/trn_rl_repo",):
    if _p not in sys.path:
        sys.path.insert(0, _p)

import numpy as np

import concourse.bass as bass
import concourse.bacc as bacc
import concourse.mybir as mybir
import concourse.tile as tile
from concourse import bass_utils
from concourse.masks import make_identity

P = 128
N_CORES = 8

F32 = mybir.dt.float32
I32 = mybir.dt.int32


# ----------------------------------------------------------------------------
# Host-side preprocessing
# ----------------------------------------------------------------------------

def _bucketize(target, gather, weight, n_nodes, nloc, n_windows):
    """Sort one edge stream by target node; return per (core, window) lists.

    Returns dict with per-core arrays after global sort plus window boundary
    indices: bounds[c][w] = (start, end) into the sorted arrays.
    """
    order = np.argsort(target, kind="stable")
    t_s = target[order]
    g_s = gather[order]
    w_s = weight[order]
    # window id of each edge (global): target // 128 within core-local space
    # core = target // nloc ; window = (target - core*nloc) // 128
    n_total_windows = N_CORES * n_windows
    core = t_s // nloc
    win = (t_s - core * nloc) // P
    gwin = core * n_windows + win
    # boundaries of each global window in the sorted list
    starts = np.searchsorted(gwin, np.arange(n_total_windows), side="left")
    ends = np.searchsorted(gwin, np.arange(n_total_windows), side="right")
    return t_s, g_s, w_s, starts, ends


def _pack_stream(t_s, g_s, w_s, starts, ends, n_windows, nloc, K):
    """Build padded per-core arrays of shape (NC, NW, K*128) for gidx (int32),
    t_local (f32), wt (f32)."""
    NC = N_CORES
    gidx = np.zeros((NC, n_windows, K * P), dtype=np.int32)
    tloc = np.zeros((NC, n_windows, K * P), dtype=np.float32)
    wt = np.zeros((NC, n_windows, K * P), dtype=np.float32)
    for c in range(NC):
        for w in range(n_windows):
            gw = c * n_windows + w
            s, e = starts[gw], ends[gw]
            n = e - s
            assert n <= K * P, f"window overflow: {n} > {K * P}"
            base = c * nloc + w * P
            gidx[c, w, :n] = g_s[s:e]
            tloc[c, w, :n] = (t_s[s:e] - base).astype(np.float32)
            wt[c, w, :n] = w_s[s:e]
    return gidx, tloc, wt


def _stream_K(starts, ends, n_windows):
    cnt = (ends - starts).reshape(N_CORES, n_windows)
    return int(np.max((cnt + P - 1) // P))


def prepare_inputs(inputs, n_nodes=None):
    """Host preprocessing: returns (cfg, in_maps list of per-core dicts)."""
    node_emb = np.ascontiguousarray(np.asarray(inputs["node_emb"], dtype=np.float32))
    N, D = node_emb.shape
    assert D == P
    assert N % N_CORES == 0
    nloc = N // N_CORES
    n_windows = (nloc + P - 1) // P
    nloc_pad = n_windows * P

    er_src = np.asarray(inputs["er_src"]).astype(np.int64)
    er_dst = np.asarray(inputs["er_dst"]).astype(np.int64)
    ee_src = np.asarray(inputs["ee_src"]).astype(np.int64)
    ee_dst = np.asarray(inputs["ee_dst"]).astype(np.int64)
    ee_w = np.asarray(inputs["ee_weight"], dtype=np.float32)
    rr_src = np.asarray(inputs["rr_src"]).astype(np.int64)
    rr_dst = np.asarray(inputs["rr_dst"]).astype(np.int64)

    # Attention stream: target=er_src, gather=er_dst (r_emb), weight=validity.
    at_t, at_g, at_w, at_s, at_e = _bucketize(
        er_src, er_dst, np.ones_like(ee_w, shape=er_src.shape), N, nloc, n_windows
    )
    KA = _stream_K(at_s, at_e, n_windows)

    # Merged mean stream: (er by dst, gather src, wt 1), (ee by src, gather
    # dst, wt ee_weight), (rr by src, gather dst, wt 1).
    m_t = np.concatenate([er_dst, ee_src, rr_src])
    m_g = np.concatenate([er_src, ee_dst, rr_dst])
    m_w = np.concatenate(
        [np.ones(er_dst.shape[0], np.float32), ee_w, np.ones(rr_src.shape[0], np.float32)]
    )
    mn_t, mn_g, mn_w, mn_s, mn_e = _bucketize(m_t, m_g, m_w, N, nloc, n_windows)
    KM = _stream_K(mn_s, mn_e, n_windows)

    a_gidx, a_t, a_v = _pack_stream(at_t, at_g, at_w, at_s, at_e, n_windows, nloc, KA)
    m_gidx, m_t2, m_w2 = _pack_stream(mn_t, mn_g, mn_w, mn_s, mn_e, n_windows, nloc, KM)

    KT = KA + KM
    # meta layout per (core, window): (128 partitions, 3*KT) int32:
    #   [:, 0:KT]      gather indices (attn tiles then mean tiles)
    #   [:, KT:2KT]    t_local as f32 bits
    #   [:, 2KT:3KT]   v / wt as f32 bits
    # partition p, col k corresponds to edge slot (tile k, lane p).
    def to_pk(a):  # (NC, NW, K*P) -> (NC, NW, P, K)
        return a.reshape(N_CORES, n_windows, -1, P).transpose(0, 1, 3, 2)

    meta = np.zeros((N_CORES, n_windows, P, 3 * KT), dtype=np.int32)
    meta[..., 0:KA] = to_pk(a_gidx)
    meta[..., KA:KT] = to_pk(m_gidx)
    meta[..., KT : KT + KA] = to_pk(a_t).view(np.int32)
    meta[..., KT + KA : 2 * KT] = to_pk(m_t2).view(np.int32)
    meta[..., 2 * KT : 2 * KT + KA] = to_pk(a_v).view(np.int32)
    meta[..., 2 * KT + KA : 3 * KT] = to_pk(m_w2).view(np.int32)

    W_attn_w = np.asarray(inputs["W_attn_w"], dtype=np.float32)  # (128, 256)
    W_attn_b = np.asarray(inputs["W_attn_b"], dtype=np.float32)  # (128,)
    w0_w = np.asarray(inputs["w0_w"], dtype=np.float32)  # (1, 128)
    w0_b = float(np.asarray(inputs["w0_b"], dtype=np.float32)[0])
    W1_w = np.asarray(inputs["W1_w"], dtype=np.float32)
    W1_b = np.asarray(inputs["W1_b"], dtype=np.float32)
    W2_w = np.asarray(inputs["W2_w"], dtype=np.float32)
    W2_b = np.asarray(inputs["W2_b"], dtype=np.float32)
    W3_w = np.asarray(inputs["W3_w"], dtype=np.float32)
    W3_b = np.asarray(inputs["W3_b"], dtype=np.float32)

    WrT = np.ascontiguousarray(W_attn_w[:, :P].T)  # (D, feat)
    WhT = np.ascontiguousarray(W_attn_w[:, P:].T)  # (D, feat)
    w0col = np.ascontiguousarray(w0_w.T)  # (feat, 1)
    ab_col = np.ascontiguousarray(W_attn_b[:, None])  # (feat, 1)
    W1T = np.ascontiguousarray(W1_w.T)
    W2T = np.ascontiguousarray(W2_w.T)
    W3T = np.ascontiguousarray(W3_w.T)
    b1r = np.ascontiguousarray(W1_b[None, :])
    b2r = np.ascontiguousarray(W2_b[None, :])
    b3r = np.ascontiguousarray(W3_b[None, :])
    ones_row = np.ones((1, P), dtype=np.float32)
    w0b_col = np.full((P, 1), w0_b, dtype=np.float32)
    iota_row = np.tile(np.arange(P, dtype=np.float32)[None, :], (P, 1))

    in_maps = []
    for c in range(N_CORES):
        sl = node_emb[c * nloc : (c + 1) * nloc]
        nodeT = np.zeros((P, nloc_pad), dtype=np.float32)
        nodeT[:, :nloc] = sl.T
        in_maps.append(
            {
                "node_emb": node_emb,
                "nodeT": nodeT,
                "meta": np.ascontiguousarray(meta[c]),
                "WrT": WrT,
                "WhT": WhT,
                "w0col": w0col,
                "ab_col": ab_col,
                "W1T": W1T,
                "W2T": W2T,
                "W3T": W3T,
                "b1r": b1r,
                "b2r": b2r,
                "b3r": b3r,
                "ones_row": ones_row,
                "w0b_col": w0b_col,
                "iota_row": iota_row,
            }
        )

    cfg = dict(
        N=N, nloc=nloc, n_windows=n_windows, nloc_pad=nloc_pad,
        KA=KA, KM=KM, KT=KT, w0_b=w0_b,
    )
    return cfg, in_maps


# ----------------------------------------------------------------------------
# Bass kernel builder
# ----------------------------------------------------------------------------

def build_bass(cfg):
    N = cfg["N"]
    NW = cfg["n_windows"]
    KA, KM, KT = cfg["KA"], cfg["KM"], cfg["KT"]
    nloc_pad = cfg["nloc_pad"]
    w0_b = cfg["w0_b"]

    nc = bacc.Bacc(trn_type="TRN2")

    node_emb = nc.dram_tensor("node_emb", [N, P], F32, kind="ExternalInput")
    nodeT = nc.dram_tensor("nodeT", [P, nloc_pad], F32, kind="ExternalInput")
    meta_d = nc.dram_tensor("meta", [NW, P, 3 * KT], I32, kind="ExternalInput")
    WrT_d = nc.dram_tensor("WrT", [P, P], F32, kind="ExternalInput")
    WhT_d = nc.dram_tensor("WhT", [P, P], F32, kind="ExternalInput")
    w0col_d = nc.dram_tensor("w0col", [P, 1], F32, kind="ExternalInput")
    ab_col_d = nc.dram_tensor("ab_col", [P, 1], F32, kind="ExternalInput")
    W1T_d = nc.dram_tensor("W1T", [P, P], F32, kind="ExternalInput")
    W2T_d = nc.dram_tensor("W2T", [P, P], F32, kind="ExternalInput")
    W3T_d = nc.dram_tensor("W3T", [P, P], F32, kind="ExternalInput")
    b1r_d = nc.dram_tensor("b1r", [1, P], F32, kind="ExternalInput")
    b2r_d = nc.dram_tensor("b2r", [1, P], F32, kind="ExternalInput")
    b3r_d = nc.dram_tensor("b3r", [1, P], F32, kind="ExternalInput")
    ones_d = nc.dram_tensor("ones_row", [1, P], F32, kind="ExternalInput")
    w0b_d = nc.dram_tensor("w0b_col", [P, 1], F32, kind="ExternalInput")
    iota_d = nc.dram_tensor("iota_row", [P, P], F32, kind="ExternalInput")

    out_d = nc.dram_tensor("out", [nloc_pad, P], F32, kind="ExternalOutput")

    with tile.TileContext(nc) as tc:
        with (
            tc.tile_pool(name="const", bufs=1) as cpool,
            tc.tile_pool(name="win", bufs=2) as wpool,
            tc.tile_pool(name="work", bufs=3) as kpool,
            tc.tile_pool(name="psacc", bufs=1, space="PSUM") as ps_acc,
            tc.tile_pool(name="pstr", bufs=4, space="PSUM") as ps_tr,
            tc.tile_pool(name="psecol", bufs=2, space="PSUM") as ps_ecol,
        ):
            # ---- constants ----
            ident = cpool.tile([P, P], F32, tag="ident")
            make_identity(nc, ident[:])
            iota_sb = cpool.tile([P, P], F32, tag="iota")
            nc.sync.dma_start(out=iota_sb[:], in_=iota_d[:, :])
            WrT_sb = cpool.tile([P, P], F32, tag="WrT")
            nc.sync.dma_start(out=WrT_sb[:], in_=WrT_d[:, :])
            WhT_sb = cpool.tile([P, P], F32, tag="WhT")
            nc.sync.dma_start(out=WhT_sb[:], in_=WhT_d[:, :])
            w0col_sb = cpool.tile([P, 1], F32, tag="w0col")
            nc.sync.dma_start(out=w0col_sb[:], in_=w0col_d[:, :])
            ab_sb = cpool.tile([P, 1], F32, tag="ab")
            nc.sync.dma_start(out=ab_sb[:], in_=ab_col_d[:, :])
            W1T_sb = cpool.tile([P, P], F32, tag="W1T")
            nc.sync.dma_start(out=W1T_sb[:], in_=W1T_d[:, :])
            W2T_sb = cpool.tile([P, P], F32, tag="W2T")
            nc.sync.dma_start(out=W2T_sb[:], in_=W2T_d[:, :])
            W3T_sb = cpool.tile([P, P], F32, tag="W3T")
            nc.sync.dma_start(out=W3T_sb[:], in_=W3T_d[:, :])
            b1_sb = cpool.tile([1, P], F32, tag="b1")
            nc.sync.dma_start(out=b1_sb[:], in_=b1r_d[:, :])
            b2_sb = cpool.tile([1, P], F32, tag="b2")
            nc.sync.dma_start(out=b2_sb[:], in_=b2r_d[:, :])
            b3_sb = cpool.tile([1, P], F32, tag="b3")
            nc.sync.dma_start(out=b3_sb[:], in_=b3r_d[:, :])
            ones_sb = cpool.tile([1, P], F32, tag="ones")
            nc.sync.dma_start(out=ones_sb[:], in_=ones_d[:, :])
            w0b_sb = cpool.tile([P, 1], F32, tag="w0b")
            nc.sync.dma_start(out=w0b_sb[:], in_=w0b_d[:, :])

            for w in range(NW):
                # ---- per-window loads ----
                meta_sb = wpool.tile([P, 3 * KT], I32, tag="meta")
                nc.sync.dma_start(out=meta_sb[:], in_=meta_d[w, :, :])
                nodeT_sb = wpool.tile([P, P], F32, tag="nodeT")
                nc.sync.dma_start(out=nodeT_sb[:], in_=nodeT[:, w * P : (w + 1) * P])

                t_all = meta_sb[:, KT : 2 * KT].bitcast(F32)
                v_all = meta_sb[:, 2 * KT : 3 * KT].bitcast(F32)

                # gathers (one indirect DMA per 128-edge tile: HW reads one
                # index per partition)
                r_big = wpool.tile([P, KA * P], F32, tag="r_big")
                for k in range(KA):
                    nc.gpsimd.indirect_dma_start(
                        out=r_big[:, k * P : (k + 1) * P],
                        out_offset=None,
                        in_=node_emb[:, :],
                        in_offset=bass.IndirectOffsetOnAxis(
                            ap=meta_sb[:, k : k + 1], axis=0
                        ),
                    )
                g_big = wpool.tile([P, KM * P], F32, tag="g_big")
                for k in range(KM):
                    nc.gpsimd.indirect_dma_start(
                        out=g_big[:, k * P : (k + 1) * P],
                        out_offset=None,
                        in_=node_emb[:, :],
                        in_offset=bass.IndirectOffsetOnAxis(
                            ap=meta_sb[:, KA + k : KA + k + 1], axis=0
                        ),
                    )

                # WH_w = window @ WhT   (nodes x feat)
                wh_ps = ps_tr.tile([P, P], F32, tag="tr")
                nc.tensor.matmul(out=wh_ps[:], lhsT=nodeT_sb[:], rhs=WhT_sb[:],
                                 start=True, stop=True)
                WH_sb = kpool.tile([P, P], F32, tag="WH")
                nc.vector.tensor_copy(out=WH_sb[:], in_=wh_ps[:])

                A_ps = ps_acc.tile([P, P + 1], F32, tag="A")
                M_ps = ps_acc.tile([P, P + 1], F32, tag="M")

                # ---- attention tiles ----
                for k in range(KA):
                    r_sb = r_big[:, k * P : (k + 1) * P]
                    # one-hot P
                    P_sb = kpool.tile([P, P], F32, tag="P")
                    nc.vector.tensor_scalar(
                        out=P_sb[:], in0=iota_sb[:],
                        scalar1=t_all[:, k : k + 1], scalar2=None,
                        op0=mybir.AluOpType.is_equal,
                    )
                    # transposes
                    rT_ps = ps_tr.tile([P, P], F32, tag="tr")
                    nc.tensor.transpose(out=rT_ps[:], in_=r_sb, identity=ident[:])
                    rT_sb = kpool.tile([P, P], F32, tag="rT")
                    nc.vector.tensor_copy(out=rT_sb[:], in_=rT_ps[:])
                    PT_ps = ps_tr.tile([P, P], F32, tag="tr")
                    nc.tensor.transpose(out=PT_ps[:], in_=P_sb[:], identity=ident[:])
                    PT_sb = kpool.tile([P, P], F32, tag="PT")
                    nc.scalar.copy(out=PT_sb[:], in_=PT_ps[:])

                    # e_preT = WrT.T @ rT + WH.T @ PT   (feat x edges)
                    eT_ps = ps_tr.tile([P, P], F32, tag="tr")
                    nc.tensor.matmul(out=eT_ps[:], lhsT=WrT_sb[:], rhs=rT_sb[:],
                                     start=True, stop=False)
                    nc.tensor.matmul(out=eT_ps[:], lhsT=WH_sb[:], rhs=PT_sb[:],
                                     start=False, stop=True)
                    tanhT_sb = kpool.tile([P, P], F32, tag="tanhT")
                    nc.scalar.activation(
                        out=tanhT_sb[:], in_=eT_ps[:],
                        func=mybir.ActivationFunctionType.Tanh, bias=ab_sb[:],
                    )
                    # e column = tanhT.T @ w0col  (edges x 1)
                    ecol_ps = ps_ecol.tile([P, 1], F32, tag="ecol")
                    nc.tensor.matmul(out=ecol_ps[:], lhsT=tanhT_sb[:], rhs=w0col_sb[:],
                                     start=True, stop=True)
                    wraw_sb = kpool.tile([P, 1], F32, tag="wraw")
                    nc.scalar.activation(
                        out=wraw_sb[:], in_=ecol_ps[:],
                        func=mybir.ActivationFunctionType.Exp, bias=w0b_sb[:],
                    )
                    wcol_sb = kpool.tile([P, 1], F32, tag="wcol")
                    nc.vector.tensor_tensor(
                        out=wcol_sb[:], in0=wraw_sb[:],
                        in1=v_all[:, k : k + 1], op=mybir.AluOpType.mult,
                    )
                    # message [w*r | w]
                    msg_sb = kpool.tile([P, P + 1], F32, tag="msg")
                    nc.scalar.activation(
                        out=msg_sb[:, 0:P], in_=r_sb,
                        func=mybir.ActivationFunctionType.Copy, scale=wcol_sb[:],
                    )
                    nc.vector.tensor_copy(out=msg_sb[:, P : P + 1], in_=wcol_sb[:])
                    nc.tensor.matmul(out=A_ps[:], lhsT=P_sb[:], rhs=msg_sb[:],
                                     start=(k == 0), stop=(k == KA - 1),
                                     skip_group_check=True)

                # ---- mean tiles ----
                for k in range(KM):
                    g_sb = g_big[:, k * P : (k + 1) * P]
                    P_sb = kpool.tile([P, P], F32, tag="P")
                    nc.vector.tensor_scalar(
                        out=P_sb[:], in0=iota_sb[:],
                        scalar1=t_all[:, KA + k : KA + k + 1], scalar2=None,
                        op0=mybir.AluOpType.is_equal,
                    )
                    msg_sb = kpool.tile([P, P + 1], F32, tag="msg")
                    nc.scalar.activation(
                        out=msg_sb[:, 0:P], in_=g_sb,
                        func=mybir.ActivationFunctionType.Copy,
                        scale=v_all[:, KA + k : KA + k + 1],
                    )
                    nc.vector.tensor_copy(
                        out=msg_sb[:, P : P + 1], in_=v_all[:, KA + k : KA + k + 1]
                    )
                    nc.tensor.matmul(out=M_ps[:], lhsT=P_sb[:], rhs=msg_sb[:],
                                     start=(k == 0), stop=(k == KM - 1),
                                     skip_group_check=True)

                # ---- normalize ----
                sden_sb = kpool.tile([P, 1], F32, tag="sden")
                nc.vector.tensor_scalar_add(sden_sb[:], A_ps[:, P : P + 1], 1e-9)
                srec_sb = kpool.tile([P, 1], F32, tag="srec")
                nc.vector.reciprocal(srec_sb[:], sden_sb[:])
                attn_sb = kpool.tile([P, P], F32, tag="attn")
                nc.scalar.activation(
                    out=attn_sb[:], in_=A_ps[:, 0:P],
                    func=mybir.ActivationFunctionType.Copy, scale=srec_sb[:],
                )
                cden_sb = kpool.tile([P, 1], F32, tag="cden")
                nc.vector.tensor_scalar_max(cden_sb[:], M_ps[:, P : P + 1], 1.0)
                crec_sb = kpool.tile([P, 1], F32, tag="crec")
                nc.vector.reciprocal(crec_sb[:], cden_sb[:])
                mean_sb = kpool.tile([P, P], F32, tag="mean")
                nc.scalar.activation(
                    out=mean_sb[:], in_=M_ps[:, 0:P],
                    func=mybir.ActivationFunctionType.Copy, scale=crec_sb[:],
                )

                # ---- final matmuls ----
                aT_ps = ps_tr.tile([P, P], F32, tag="tr")
                nc.tensor.transpose(out=aT_ps[:], in_=attn_sb[:], identity=ident[:])
                aT_sb = kpool.tile([P, P], F32, tag="aT")
                nc.vector.tensor_copy(out=aT_sb[:], in_=aT_ps[:])
                mT_ps = ps_tr.tile([P, P], F32, tag="tr")
                nc.tensor.transpose(out=mT_ps[:], in_=mean_sb[:], identity=ident[:])
                mT_sb = kpool.tile([P, P], F32, tag="mT")
                nc.vector.tensor_copy(out=mT_sb[:], in_=mT_ps[:])

                out_sb = kpool.tile([P, P], F32, tag="out")
                acc_sb = kpool.tile([P, P], F32, tag="acc")
                for term, (lhsT_sb, Wt_sb, b_sb) in enumerate(
                    [
                        (nodeT_sb, W1T_sb, b1_sb),
                        (aT_sb, W2T_sb, b2_sb),
                        (mT_sb, W3T_sb, b3_sb),
                    ]
                ):
                    O_ps = ps_tr.tile([P, P], F32, tag="tr")
                    nc.tensor.matmul(out=O_ps[:], lhsT=ones_sb[:], rhs=b_sb[:],
                                     start=True, stop=False, skip_group_check=True)
                    nc.tensor.matmul(out=O_ps[:], lhsT=lhsT_sb[:], rhs=Wt_sb[:],
                                     start=False, stop=True, skip_group_check=True)
                    if term == 0:
                        nc.scalar.activation(
                            out=acc_sb[:], in_=O_ps[:],
                            func=mybir.ActivationFunctionType.Tanh,
                        )
                    else:
                        t_sb = kpool.tile([P, P], F32, tag="tterm")
                        nc.scalar.activation(
                            out=t_sb[:], in_=O_ps[:],
                            func=mybir.ActivationFunctionType.Tanh,
                        )
                        nc.vector.tensor_add(
                            out=(out_sb[:] if term == 2 else acc_sb[:]),
                            in0=acc_sb[:], in1=t_sb[:],
                        )
                nc.sync.dma_start(out=out_d[w * P : (w + 1) * P, :], in_=out_sb[:])

    nc.compile()
    return nc


# ----------------------------------------------------------------------------
# Entry point
# ----------------------------------------------------------------------------

_CACHE = {}


def _get_nc(cfg):
    key = tuple(sorted((k, v) for k, v in cfg.items()))
    if key not in _CACHE:
        _CACHE[key] = build_bass(cfg)
    return _CACHE[key]


def kernel(**inputs):
    cfg, in_maps = prepare_inputs(inputs)
    nc = _get_nc(cfg)
    res = bass_utils.run_bass_kernel_spmd(nc, in_maps, core_ids=list(range(N_CORES)))
    nloc = cfg["nloc"]
    outs = [r["out"][:nloc] for r in res.results]
    return np.ascontiguousarray(np.concatenate(outs, axis=0), dtype=np.float32)


# revision 11
# speedup vs baseline: 1.0072x; 1.0072x over previous
"""Trainium2 Bass kernel for AttentionGNNLayer (8-core SPMD).

Strategy: shard destination nodes across 8 cores (12500 each). Host buckets
edges by aggregation target into 128-node windows; device gathers source
embeddings via batched indirect DMA and scatter-adds via one-hot matmuls into
PSUM. Segment softmax is computed without max-subtraction (logits are
tanh-bounded), so numerator and denominator come out of one accumulation pass.
"""

import sys

for _p in ("/opt/trn_rl_repo",):
    if _p not in sys.path:
        sys.path.insert(0, _p)

import numpy as np

import concourse.bass as bass
import concourse.bacc as bacc
import concourse.mybir as mybir
import concourse.tile as tile
from concourse import bass_utils
from concourse.library_config import mlp
from concourse.masks import make_identity

P = 128
N_CORES = 8

F32 = mybir.dt.float32
I32 = mybir.dt.int32


# ----------------------------------------------------------------------------
# Host-side preprocessing
# ----------------------------------------------------------------------------

def _bucketize(target, gather, weight, n_nodes, nloc, n_windows):
    """Sort one edge stream by target node; return per (core, window) lists.

    Returns dict with per-core arrays after global sort plus window boundary
    indices: bounds[c][w] = (start, end) into the sorted arrays.
    """
    order = np.argsort(target, kind="stable")
    t_s = target[order]
    g_s = gather[order]
    w_s = weight[order]
    # window id of each edge (global): target // 128 within core-local space
    # core = target // nloc ; window = (target - core*nloc) // 128
    n_total_windows = N_CORES * n_windows
    core = t_s // nloc
    win = (t_s - core * nloc) // P
    gwin = core * n_windows + win
    # boundaries of each global window in the sorted list
    starts = np.searchsorted(gwin, np.arange(n_total_windows), side="left")
    ends = np.searchsorted(gwin, np.arange(n_total_windows), side="right")
    return t_s, g_s, w_s, starts, ends


def _region_counts(t_s, g_s, starts, ends, n_windows, region_size):
    """Per (core, window, region) tile counts; returns (NC, NW, 4) int array."""
    cnt = np.zeros((N_CORES, n_windows, 4), dtype=np.int64)
    reg = g_s // region_size
    for c in range(N_CORES):
        for w in range(n_windows):
            gw = c * n_windows + w
            s, e = starts[gw], ends[gw]
            r = reg[s:e]
            for rr in range(4):
                cnt[c, w, rr] = int(np.sum(r == rr))
    return cnt


def _pack_stream_regions(t_s, g_s, w_s, starts, ends, n_windows, nloc, Ks,
                         region_size):
    """Region-pure padded packing.

    Ks: list of 4 per-region tile counts (uniform across cores/windows).
    Returns gidx16 (NC, NW, Ktot*128) int16 (region-relative), tloc f32,
    wt f32 arrays in slot order [region0 tiles | region1 | ...].
    """
    NC = N_CORES
    Ktot = int(sum(Ks))
    offs = np.concatenate([[0], np.cumsum([k * P for k in Ks])]).astype(np.int64)
    gidx = np.zeros((NC, n_windows, Ktot * P), dtype=np.int16)
    tloc = np.zeros((NC, n_windows, Ktot * P), dtype=np.float32)
    wt = np.zeros((NC, n_windows, Ktot * P), dtype=np.float32)
    reg = g_s // region_size
    for c in range(NC):
        for w in range(n_windows):
            gw = c * n_windows + w
            s, e = starts[gw], ends[gw]
            base = c * nloc + w * P
            r_all = reg[s:e]
            for rr in range(4):
                m = r_all == rr
                n = int(np.sum(m))
                assert n <= Ks[rr] * P, f"region overflow {n} > {Ks[rr]*P}"
                o = offs[rr]
                gidx[c, w, o : o + n] = (g_s[s:e][m] - rr * region_size).astype(np.int16)
                tloc[c, w, o : o + n] = (t_s[s:e][m] - base).astype(np.float32)
                wt[c, w, o : o + n] = w_s[s:e][m]
    return gidx, tloc, wt


def _wrap_idx16(gidx_pk):
    """(NC, NW, Ktot*P) slot-ordered int16 -> wrapped layout (NC, NW, P, Ktot*8).

    dma_gather expects idx j of a call at partition (16g + j%16), free j//16,
    replicated for g in 0..7. Calls slice along the free dim, so wrap the
    whole slot array at once (call boundaries are multiples of 128 slots =
    8 free columns)."""
    NC, NW, S = gidx_pk.shape
    w = gidx_pk.reshape(NC, NW, S // 16, 16).transpose(0, 1, 3, 2)  # (NC,NW,16,S/16)
    return np.ascontiguousarray(
        np.broadcast_to(w[:, :, None, :, :], (NC, NW, 8, 16, S // 16)).reshape(
            NC, NW, P, S // 16
        )
    )


def prepare_inputs(inputs, n_nodes=None):
    """Host preprocessing: returns (cfg, in_maps list of per-core dicts)."""
    node_emb = np.ascontiguousarray(np.asarray(inputs["node_emb"], dtype=np.float32))
    N, D = node_emb.shape
    assert D == P
    assert N % N_CORES == 0
    nloc = N // N_CORES
    n_windows = (nloc + P - 1) // P
    nloc_pad = n_windows * P

    er_src = np.asarray(inputs["er_src"]).astype(np.int64)
    er_dst = np.asarray(inputs["er_dst"]).astype(np.int64)
    ee_src = np.asarray(inputs["ee_src"]).astype(np.int64)
    ee_dst = np.asarray(inputs["ee_dst"]).astype(np.int64)
    ee_w = np.asarray(inputs["ee_weight"], dtype=np.float32)
    rr_src = np.asarray(inputs["rr_src"]).astype(np.int64)
    rr_dst = np.asarray(inputs["rr_dst"]).astype(np.int64)

    # int16 gather regions (4 equal, 128-aligned)
    region_size = ((N + 4 * P - 1) // (4 * P)) * P
    assert region_size <= 32767, region_size

    # Attention stream: target=er_src, gather=er_dst (r_emb), weight=validity.
    at_t, at_g, at_w, at_s, at_e = _bucketize(
        er_src, er_dst, np.ones_like(ee_w, shape=er_src.shape), N, nloc, n_windows
    )
    at_cnt = _region_counts(at_t, at_g, at_s, at_e, n_windows, region_size)
    KAr = [int(np.max((at_cnt[..., r] + P - 1) // P)) for r in range(4)]
    KA = int(sum(KAr))

    # Merged mean stream: (er by dst, gather src, wt 1), (ee by src, gather
    # dst, wt ee_weight), (rr by src, gather dst, wt 1).
    m_t = np.concatenate([er_dst, ee_src, rr_src])
    m_g = np.concatenate([er_src, ee_dst, rr_dst])
    m_w = np.concatenate(
        [np.ones(er_dst.shape[0], np.float32), ee_w, np.ones(rr_src.shape[0], np.float32)]
    )
    mn_t, mn_g, mn_w, mn_s, mn_e = _bucketize(m_t, m_g, m_w, N, nloc, n_windows)
    mn_cnt = _region_counts(mn_t, mn_g, mn_s, mn_e, n_windows, region_size)
    KMr = [int(np.max((mn_cnt[..., r] + P - 1) // P)) for r in range(4)]
    KM = int(sum(KMr))

    a_g16, a_t, a_v = _pack_stream_regions(
        at_t, at_g, at_w, at_s, at_e, n_windows, nloc, KAr, region_size)
    m_g16, m_t2, m_w2 = _pack_stream_regions(
        mn_t, mn_g, mn_w, mn_s, mn_e, n_windows, nloc, KMr, region_size)

    KT = KA + KM
    # meta layout per (core, window): (128 partitions, 2*KT) int32:
    #   [:, 0:KT]    t_local as f32 bits (attn tiles then mean tiles)
    #   [:, KT:2KT]  v / wt as f32 bits
    # partition p, col k corresponds to edge slot (tile k, lane p).
    def to_pk(a):  # (NC, NW, K*P) -> (NC, NW, P, K)
        return a.reshape(N_CORES, n_windows, -1, P).transpose(0, 1, 3, 2)

    meta = np.zeros((N_CORES, n_windows, P, 2 * KT), dtype=np.int32)
    meta[..., 0:KA] = to_pk(a_t).view(np.int32)
    meta[..., KA:KT] = to_pk(m_t2).view(np.int32)
    meta[..., KT : KT + KA] = to_pk(a_v).view(np.int32)
    meta[..., KT + KA : 2 * KT] = to_pk(m_w2).view(np.int32)

    # wrapped int16 gather indices: (NC, NW, P, 8*KT)
    idx16 = np.concatenate([_wrap_idx16(a_g16), _wrap_idx16(m_g16)], axis=3)

    W_attn_w = np.asarray(inputs["W_attn_w"], dtype=np.float32)  # (128, 256)
    W_attn_b = np.asarray(inputs["W_attn_b"], dtype=np.float32)  # (128,)
    w0_w = np.asarray(inputs["w0_w"], dtype=np.float32)  # (1, 128)
    w0_b = float(np.asarray(inputs["w0_b"], dtype=np.float32)[0])
    W1_w = np.asarray(inputs["W1_w"], dtype=np.float32)
    W1_b = np.asarray(inputs["W1_b"], dtype=np.float32)
    W2_w = np.asarray(inputs["W2_w"], dtype=np.float32)
    W2_b = np.asarray(inputs["W2_b"], dtype=np.float32)
    W3_w = np.asarray(inputs["W3_w"], dtype=np.float32)
    W3_b = np.asarray(inputs["W3_b"], dtype=np.float32)

    WrT = np.ascontiguousarray(W_attn_w[:, :P].T)  # (D, feat)
    WhT = np.ascontiguousarray(W_attn_w[:, P:].T)  # (D, feat)
    w0col = np.ascontiguousarray(w0_w.T)  # (feat, 1)
    ab_col = np.ascontiguousarray(W_attn_b[:, None])  # (feat, 1)
    W1T = np.ascontiguousarray(W1_w.T)
    W2T = np.ascontiguousarray(W2_w.T)
    W3T = np.ascontiguousarray(W3_w.T)
    b1r = np.ascontiguousarray(W1_b[None, :])
    b2r = np.ascontiguousarray(W2_b[None, :])
    b3r = np.ascontiguousarray(W3_b[None, :])
    ones_row = np.ones((1, P), dtype=np.float32)
    w0b_col = np.full((P, 1), w0_b, dtype=np.float32)
    iota_row = np.tile(np.arange(P, dtype=np.float32)[None, :], (P, 1))

    in_maps = []
    for c in range(N_CORES):
        sl = node_emb[c * nloc : (c + 1) * nloc]
        nodeT = np.zeros((P, nloc_pad), dtype=np.float32)
        nodeT[:, :nloc] = sl.T
        in_maps.append(
            {
                "node_emb": node_emb,
                "nodeT": nodeT,
                "meta": np.ascontiguousarray(meta[c]),
                "idx16": np.ascontiguousarray(idx16[c]),
                "WrT": WrT,
                "WhT": WhT,
                "w0col": w0col,
                "ab_col": ab_col,
                "W1T": W1T,
                "W2T": W2T,
                "W3T": W3T,
                "b1r": b1r,
                "b2r": b2r,
                "b3r": b3r,
                "ones_row": ones_row,
                "w0b_col": w0b_col,
                "iota_row": iota_row,
            }
        )

    cfg = dict(
        N=N, nloc=nloc, n_windows=n_windows, nloc_pad=nloc_pad,
        KA=KA, KM=KM, KT=KT, w0_b=w0_b,
        KAr=tuple(KAr), KMr=tuple(KMr), region_size=region_size,
    )
    return cfg, in_maps


# ----------------------------------------------------------------------------
# Bass kernel builder
# ----------------------------------------------------------------------------

def build_bass(cfg):
    N = cfg["N"]
    NW = cfg["n_windows"]
    KA, KM, KT = cfg["KA"], cfg["KM"], cfg["KT"]
    KAr, KMr = list(cfg["KAr"]), list(cfg["KMr"])
    region_size = cfg["region_size"]
    nloc_pad = cfg["nloc_pad"]
    w0_b = cfg["w0_b"]

    nc = bacc.Bacc(trn_type="TRN2", num_swdge_queues=4)

    node_emb = nc.dram_tensor("node_emb", [N, P], F32, kind="ExternalInput")
    nodeT = nc.dram_tensor("nodeT", [P, nloc_pad], F32, kind="ExternalInput")
    meta_d = nc.dram_tensor("meta", [NW, P, 2 * KT], I32, kind="ExternalInput")
    idx16_d = nc.dram_tensor("idx16", [NW, P, 8 * KT], mybir.dt.int16, kind="ExternalInput")
    WrT_d = nc.dram_tensor("WrT", [P, P], F32, kind="ExternalInput")
    WhT_d = nc.dram_tensor("WhT", [P, P], F32, kind="ExternalInput")
    w0col_d = nc.dram_tensor("w0col", [P, 1], F32, kind="ExternalInput")
    ab_col_d = nc.dram_tensor("ab_col", [P, 1], F32, kind="ExternalInput")
    W1T_d = nc.dram_tensor("W1T", [P, P], F32, kind="ExternalInput")
    W2T_d = nc.dram_tensor("W2T", [P, P], F32, kind="ExternalInput")
    W3T_d = nc.dram_tensor("W3T", [P, P], F32, kind="ExternalInput")
    b1r_d = nc.dram_tensor("b1r", [1, P], F32, kind="ExternalInput")
    b2r_d = nc.dram_tensor("b2r", [1, P], F32, kind="ExternalInput")
    b3r_d = nc.dram_tensor("b3r", [1, P], F32, kind="ExternalInput")
    ones_d = nc.dram_tensor("ones_row", [1, P], F32, kind="ExternalInput")
    w0b_d = nc.dram_tensor("w0b_col", [P, 1], F32, kind="ExternalInput")
    iota_d = nc.dram_tensor("iota_row", [P, P], F32, kind="ExternalInput")

    out_d = nc.dram_tensor("out", [nloc_pad, P], F32, kind="ExternalOutput")

    with tile.TileContext(nc) as tc:
        with (
            tc.tile_pool(name="const", bufs=1) as cpool,
            tc.tile_pool(name="win", bufs=2) as wpool,
            tc.tile_pool(name="work", bufs=3) as kpool,
            tc.tile_pool(name="psacc", bufs=1, space="PSUM") as ps_acc,
            tc.tile_pool(name="pstr", bufs=4, space="PSUM") as ps_tr,
            tc.tile_pool(name="psecol", bufs=2, space="PSUM") as ps_ecol,
        ):
            # ---- constants ----
            nc.gpsimd.load_library(mlp)
            ident = cpool.tile([P, P], F32, tag="ident")
            make_identity(nc, ident[:])
            iota_sb = cpool.tile([P, P], F32, tag="iota")
            nc.sync.dma_start(out=iota_sb[:], in_=iota_d[:, :])
            WrT_sb = cpool.tile([P, P], F32, tag="WrT")
            nc.sync.dma_start(out=WrT_sb[:], in_=WrT_d[:, :])
            WhT_sb = cpool.tile([P, P], F32, tag="WhT")
            nc.sync.dma_start(out=WhT_sb[:], in_=WhT_d[:, :])
            w0col_sb = cpool.tile([P, 1], F32, tag="w0col")
            nc.sync.dma_start(out=w0col_sb[:], in_=w0col_d[:, :])
            ab_sb = cpool.tile([P, 1], F32, tag="ab")
            nc.sync.dma_start(out=ab_sb[:], in_=ab_col_d[:, :])
            W1T_sb = cpool.tile([P, P], F32, tag="W1T")
            nc.sync.dma_start(out=W1T_sb[:], in_=W1T_d[:, :])
            W2T_sb = cpool.tile([P, P], F32, tag="W2T")
            nc.sync.dma_start(out=W2T_sb[:], in_=W2T_d[:, :])
            W3T_sb = cpool.tile([P, P], F32, tag="W3T")
            nc.sync.dma_start(out=W3T_sb[:], in_=W3T_d[:, :])
            b1_sb = cpool.tile([1, P], F32, tag="b1")
            nc.sync.dma_start(out=b1_sb[:], in_=b1r_d[:, :])
            b2_sb = cpool.tile([1, P], F32, tag="b2")
            nc.sync.dma_start(out=b2_sb[:], in_=b2r_d[:, :])
            b3_sb = cpool.tile([1, P], F32, tag="b3")
            nc.sync.dma_start(out=b3_sb[:], in_=b3r_d[:, :])
            ones_sb = cpool.tile([1, P], F32, tag="ones")
            nc.sync.dma_start(out=ones_sb[:], in_=ones_d[:, :])
            w0b_sb = cpool.tile([P, 1], F32, tag="w0b")
            nc.sync.dma_start(out=w0b_sb[:], in_=w0b_d[:, :])

            for w in range(NW):
                # ---- per-window loads ----
                meta_sb = wpool.tile([P, 2 * KT], I32, tag="meta")
                nc.sync.dma_start(out=meta_sb[:], in_=meta_d[w, :, :])
                idx_sb = wpool.tile([P, 8 * KT], mybir.dt.int16, tag="idx16")
                nc.sync.dma_start(out=idx_sb[:], in_=idx16_d[w, :, :])
                nodeT_sb = wpool.tile([P, P], F32, tag="nodeT")
                nc.sync.dma_start(out=nodeT_sb[:], in_=nodeT[:, w * P : (w + 1) * P])

                t_all = meta_sb[:, 0:KT].bitcast(F32)
                v_all = meta_sb[:, KT : 2 * KT].bitcast(F32)

                # region-pure dma_gather calls (int16 idx, 4 SWDGE queues)
                r_big = wpool.tile([P, KA * P], F32, tag="r_big")
                g_big = wpool.tile([P, KM * P], F32, tag="g_big")
                qn = 0
                for big, Ks, slot0 in ((r_big, KAr, 0), (g_big, KMr, KA)):
                    off = 0
                    for r in range(4):
                        Kr = Ks[r]
                        if Kr == 0:
                            continue
                        ni = Kr * P
                        base = r * region_size
                        hi = min(N - base, region_size)
                        nc.gpsimd.dma_gather(
                            big[:, off * P : (off + Kr) * P].rearrange(
                                "p (t e) -> p t e", e=P
                            ),
                            node_emb[base : base + hi, :],
                            idx_sb[:, (slot0 + off) * 8 : (slot0 + off + Kr) * 8],
                            ni,
                            ni,
                            P,
                            queue_num=qn % 4,
                        )
                        qn += 1
                        off += Kr

                # WH_w = window @ WhT   (nodes x feat)
                wh_ps = ps_tr.tile([P, P], F32, tag="tr")
                nc.tensor.matmul(out=wh_ps[:], lhsT=nodeT_sb[:], rhs=WhT_sb[:],
                                 start=True, stop=True)
                WH_sb = kpool.tile([P, P], F32, tag="WH")
                nc.vector.tensor_copy(out=WH_sb[:], in_=wh_ps[:])

                A_ps = ps_acc.tile([P, P + 1], F32, tag="A")
                M_ps = ps_acc.tile([P, P + 1], F32, tag="M")

                # ---- attention tiles ----
                for k in range(KA):
                    r_sb = r_big[:, k * P : (k + 1) * P]
                    # one-hot P
                    P_sb = kpool.tile([P, P], F32, tag="P")
                    nc.vector.tensor_scalar(
                        out=P_sb[:], in0=iota_sb[:],
                        scalar1=t_all[:, k : k + 1], scalar2=None,
                        op0=mybir.AluOpType.is_equal,
                    )
                    # transposes
                    rT_ps = ps_tr.tile([P, P], F32, tag="tr")
                    nc.tensor.transpose(out=rT_ps[:], in_=r_sb, identity=ident[:])
                    rT_sb = kpool.tile([P, P], F32, tag="rT")
                    nc.vector.tensor_copy(out=rT_sb[:], in_=rT_ps[:])
                    PT_ps = ps_tr.tile([P, P], F32, tag="tr")
                    nc.tensor.transpose(out=PT_ps[:], in_=P_sb[:], identity=ident[:])
                    PT_sb = kpool.tile([P, P], F32, tag="PT")
                    nc.scalar.copy(out=PT_sb[:], in_=PT_ps[:])

                    # e_preT = WrT.T @ rT + WH.T @ PT   (feat x edges)
                    eT_ps = ps_tr.tile([P, P], F32, tag="tr")
                    nc.tensor.matmul(out=eT_ps[:], lhsT=WrT_sb[:], rhs=rT_sb[:],
                                     start=True, stop=False)
                    nc.tensor.matmul(out=eT_ps[:], lhsT=WH_sb[:], rhs=PT_sb[:],
                                     start=False, stop=True)
                    tanhT_sb = kpool.tile([P, P], F32, tag="tanhT")
                    nc.scalar.activation(
                        out=tanhT_sb[:], in_=eT_ps[:],
                        func=mybir.ActivationFunctionType.Tanh, bias=ab_sb[:],
                    )
                    # e column = tanhT.T @ w0col  (edges x 1)
                    ecol_ps = ps_ecol.tile([P, 1], F32, tag="ecol")
                    nc.tensor.matmul(out=ecol_ps[:], lhsT=tanhT_sb[:], rhs=w0col_sb[:],
                                     start=True, stop=True)
                    wraw_sb = kpool.tile([P, 1], F32, tag="wraw")
                    nc.scalar.activation(
                        out=wraw_sb[:], in_=ecol_ps[:],
                        func=mybir.ActivationFunctionType.Exp, bias=w0b_sb[:],
                    )
                    wcol_sb = kpool.tile([P, 1], F32, tag="wcol")
                    nc.vector.tensor_tensor(
                        out=wcol_sb[:], in0=wraw_sb[:],
                        in1=v_all[:, k : k + 1], op=mybir.AluOpType.mult,
                    )
                    # message [w*r | w]
                    msg_sb = kpool.tile([P, P + 1], F32, tag="msg")
                    nc.scalar.activation(
                        out=msg_sb[:, 0:P], in_=r_sb,
                        func=mybir.ActivationFunctionType.Copy, scale=wcol_sb[:],
                    )
                    nc.vector.tensor_copy(out=msg_sb[:, P : P + 1], in_=wcol_sb[:])
                    nc.tensor.matmul(out=A_ps[:], lhsT=P_sb[:], rhs=msg_sb[:],
                                     start=(k == 0), stop=(k == KA - 1),
                                     skip_group_check=True)

                # ---- mean tiles ----
                for k in range(KM):
                    g_sb = g_big[:, k * P : (k + 1) * P]
                    P_sb = kpool.tile([P, P], F32, tag="P")
                    nc.vector.tensor_scalar(
                        out=P_sb[:], in0=iota_sb[:],
                        scalar1=t_all[:, KA + k : KA + k + 1], scalar2=None,
                        op0=mybir.AluOpType.is_equal,
                    )
                    msg_sb = kpool.tile([P, P + 1], F32, tag="msg")
                    nc.scalar.activation(
                        out=msg_sb[:, 0:P], in_=g_sb,
                        func=mybir.ActivationFunctionType.Copy,
                        scale=v_all[:, KA + k : KA + k + 1],
                    )
                    nc.vector.tensor_copy(
                        out=msg_sb[:, P : P + 1], in_=v_all[:, KA + k : KA + k + 1]
                    )
                    nc.tensor.matmul(out=M_ps[:], lhsT=P_sb[:], rhs=msg_sb[:],
                                     start=(k == 0), stop=(k == KM - 1),
                                     skip_group_check=True)

                # ---- normalize ----
                sden_sb = kpool.tile([P, 1], F32, tag="sden")
                nc.vector.tensor_scalar_add(sden_sb[:], A_ps[:, P : P + 1], 1e-9)
                srec_sb = kpool.tile([P, 1], F32, tag="srec")
                nc.vector.reciprocal(srec_sb[:], sden_sb[:])
                attn_sb = kpool.tile([P, P], F32, tag="attn")
                nc.scalar.activation(
                    out=attn_sb[:], in_=A_ps[:, 0:P],
                    func=mybir.ActivationFunctionType.Copy, scale=srec_sb[:],
                )
                cden_sb = kpool.tile([P, 1], F32, tag="cden")
                nc.vector.tensor_scalar_max(cden_sb[:], M_ps[:, P : P + 1], 1.0)
                crec_sb = kpool.tile([P, 1], F32, tag="crec")
                nc.vector.reciprocal(crec_sb[:], cden_sb[:])
                mean_sb = kpool.tile([P, P], F32, tag="mean")
                nc.scalar.activation(
                    out=mean_sb[:], in_=M_ps[:, 0:P],
                    func=mybir.ActivationFunctionType.Copy, scale=crec_sb[:],
                )

                # ---- final matmuls ----
                aT_ps = ps_tr.tile([P, P], F32, tag="tr")
                nc.tensor.transpose(out=aT_ps[:], in_=attn_sb[:], identity=ident[:])
                aT_sb = kpool.tile([P, P], F32, tag="aT")
                nc.vector.tensor_copy(out=aT_sb[:], in_=aT_ps[:])
                mT_ps = ps_tr.tile([P, P], F32, tag="tr")
                nc.tensor.transpose(out=mT_ps[:], in_=mean_sb[:], identity=ident[:])
                mT_sb = kpool.tile([P, P], F32, tag="mT")
                nc.vector.tensor_copy(out=mT_sb[:], in_=mT_ps[:])

                out_sb = kpool.tile([P, P], F32, tag="out")
                acc_sb = kpool.tile([P, P], F32, tag="acc")
                for term, (lhsT_sb, Wt_sb, b_sb) in enumerate(
                    [
                        (nodeT_sb, W1T_sb, b1_sb),
                        (aT_sb, W2T_sb, b2_sb),
                        (mT_sb, W3T_sb, b3_sb),
                    ]
                ):
                    O_ps = ps_tr.tile([P, P], F32, tag="tr")
                    nc.tensor.matmul(out=O_ps[:], lhsT=ones_sb[:], rhs=b_sb[:],
                                     start=True, stop=False, skip_group_check=True)
                    nc.tensor.matmul(out=O_ps[:], lhsT=lhsT_sb[:], rhs=Wt_sb[:],
                                     start=False, stop=True, skip_group_check=True)
                    if term == 0:
                        nc.scalar.activation(
                            out=acc_sb[:], in_=O_ps[:],
                            func=mybir.ActivationFunctionType.Tanh,
                        )
                    else:
                        t_sb = kpool.tile([P, P], F32, tag="tterm")
                        nc.scalar.activation(
                            out=t_sb[:], in_=O_ps[:],
                            func=mybir.ActivationFunctionType.Tanh,
                        )
                        nc.vector.tensor_add(
                            out=(out_sb[:] if term == 2 else acc_sb[:]),
                            in0=acc_sb[:], in1=t_sb[:],
                        )
                nc.sync.dma_start(out=out_d[w * P : (w + 1) * P, :], in_=out_sb[:])

    nc.compile()
    return nc


# ----------------------------------------------------------------------------
# Entry point
# ----------------------------------------------------------------------------

_CACHE = {}


def _get_nc(cfg):
    key = tuple(sorted((k, v) for k, v in cfg.items()))
    if key not in _CACHE:
        _CACHE[key] = build_bass(cfg)
    return _CACHE[key]


def kernel(**inputs):
    cfg, in_maps = prepare_inputs(inputs)
    nc = _get_nc(cfg)
    res = bass_utils.run_bass_kernel_spmd(nc, in_maps, core_ids=list(range(N_CORES)))
    nloc = cfg["nloc"]
    outs = [r["out"][:nloc] for r in res.results]
    return np.ascontiguousarray(np.concatenate(outs, axis=0), dtype=np.float32)


# revision 13
# speedup vs baseline: 1.1878x; 1.1793x over previous
"""Trainium2 Bass kernel for AttentionGNNLayer (8-core SPMD).

Strategy: shard destination nodes across 8 cores (12500 each). Host buckets
edges by aggregation target into 128-node windows; device gathers source
embeddings via batched indirect DMA and scatter-adds via one-hot matmuls into
PSUM. Segment softmax is computed without max-subtraction (logits are
tanh-bounded), so numerator and denominator come out of one accumulation pass.
"""

import sys

for _p in ("/opt/trn_rl_repo",):
    if _p not in sys.path:
        sys.path.insert(0, _p)

import numpy as np

import concourse.bass as bass
import concourse.bacc as bacc
import concourse.mybir as mybir
import concourse.tile as tile
from concourse import bass_utils
from concourse.library_config import mlp
from concourse.masks import make_identity

P = 128
N_CORES = 8

F32 = mybir.dt.float32
I32 = mybir.dt.int32


# ----------------------------------------------------------------------------
# Host-side preprocessing
# ----------------------------------------------------------------------------

def _bucketize(target, gather, weight, n_nodes, nloc, n_windows):
    """Sort one edge stream by target node; return per (core, window) lists.

    Returns dict with per-core arrays after global sort plus window boundary
    indices: bounds[c][w] = (start, end) into the sorted arrays.
    """
    order = np.argsort(target, kind="stable")
    t_s = target[order]
    g_s = gather[order]
    w_s = weight[order]
    # window id of each edge (global): target // 128 within core-local space
    # core = target // nloc ; window = (target - core*nloc) // 128
    n_total_windows = N_CORES * n_windows
    core = t_s // nloc
    win = (t_s - core * nloc) // P
    gwin = core * n_windows + win
    # boundaries of each global window in the sorted list
    starts = np.searchsorted(gwin, np.arange(n_total_windows), side="left")
    ends = np.searchsorted(gwin, np.arange(n_total_windows), side="right")
    return t_s, g_s, w_s, starts, ends


def _region_counts(t_s, g_s, starts, ends, n_windows, region_size):
    """Per (core, window, region) tile counts; returns (NC, NW, 4) int array."""
    cnt = np.zeros((N_CORES, n_windows, 4), dtype=np.int64)
    reg = g_s // region_size
    for c in range(N_CORES):
        for w in range(n_windows):
            gw = c * n_windows + w
            s, e = starts[gw], ends[gw]
            r = reg[s:e]
            for rr in range(4):
                cnt[c, w, rr] = int(np.sum(r == rr))
    return cnt


def _pack_stream_regions(t_s, g_s, w_s, starts, ends, n_windows, nloc, Ks,
                         region_size):
    """Region-pure padded packing.

    Ks: list of 4 per-region tile counts (uniform across cores/windows).
    Returns gidx16 (NC, NW, Ktot*128) int16 (region-relative), tloc f32,
    wt f32 arrays in slot order [region0 tiles | region1 | ...].
    """
    NC = N_CORES
    Ktot = int(sum(Ks))
    offs = np.concatenate([[0], np.cumsum([k * P for k in Ks])]).astype(np.int64)
    gidx = np.zeros((NC, n_windows, Ktot * P), dtype=np.int16)
    tloc = np.zeros((NC, n_windows, Ktot * P), dtype=np.float32)
    wt = np.zeros((NC, n_windows, Ktot * P), dtype=np.float32)
    reg = g_s // region_size
    for c in range(NC):
        for w in range(n_windows):
            gw = c * n_windows + w
            s, e = starts[gw], ends[gw]
            base = c * nloc + w * P
            r_all = reg[s:e]
            for rr in range(4):
                m = r_all == rr
                n = int(np.sum(m))
                assert n <= Ks[rr] * P, f"region overflow {n} > {Ks[rr]*P}"
                o = offs[rr]
                gidx[c, w, o : o + n] = (g_s[s:e][m] - rr * region_size).astype(np.int16)
                tloc[c, w, o : o + n] = (t_s[s:e][m] - base).astype(np.float32)
                wt[c, w, o : o + n] = w_s[s:e][m]
    return gidx, tloc, wt


def _wrap_idx16(gidx_pk):
    """(NC, NW, Ktot*P) slot-ordered int16 -> wrapped layout (NC, NW, P, Ktot*8).

    dma_gather expects idx j of a call at partition (16g + j%16), free j//16,
    replicated for g in 0..7. Calls slice along the free dim, so wrap the
    whole slot array at once (call boundaries are multiples of 128 slots =
    8 free columns)."""
    NC, NW, S = gidx_pk.shape
    w = gidx_pk.reshape(NC, NW, S // 16, 16).transpose(0, 1, 3, 2)  # (NC,NW,16,S/16)
    return np.ascontiguousarray(
        np.broadcast_to(w[:, :, None, :, :], (NC, NW, 8, 16, S // 16)).reshape(
            NC, NW, P, S // 16
        )
    )


def prepare_inputs(inputs, n_nodes=None):
    """Host preprocessing: returns (cfg, in_maps list of per-core dicts)."""
    node_emb = np.ascontiguousarray(np.asarray(inputs["node_emb"], dtype=np.float32))
    N, D = node_emb.shape
    assert D == P
    assert N % N_CORES == 0
    nloc = N // N_CORES
    n_windows = (nloc + P - 1) // P
    nloc_pad = n_windows * P

    er_src = np.asarray(inputs["er_src"]).astype(np.int64)
    er_dst = np.asarray(inputs["er_dst"]).astype(np.int64)
    ee_src = np.asarray(inputs["ee_src"]).astype(np.int64)
    ee_dst = np.asarray(inputs["ee_dst"]).astype(np.int64)
    ee_w = np.asarray(inputs["ee_weight"], dtype=np.float32)
    rr_src = np.asarray(inputs["rr_src"]).astype(np.int64)
    rr_dst = np.asarray(inputs["rr_dst"]).astype(np.int64)

    # int16 gather regions (4 equal, 128-aligned)
    region_size = ((N + 4 * P - 1) // (4 * P)) * P
    assert region_size <= 32767, region_size

    # Attention stream: target=er_src, gather=er_dst (r_emb), weight=validity.
    at_t, at_g, at_w, at_s, at_e = _bucketize(
        er_src, er_dst, np.ones_like(ee_w, shape=er_src.shape), N, nloc, n_windows
    )
    at_cnt = _region_counts(at_t, at_g, at_s, at_e, n_windows, region_size)
    KAr = [int(np.max((at_cnt[..., r] + P - 1) // P)) for r in range(4)]
    KA = int(sum(KAr))

    # Merged mean stream: (er by dst, gather src, wt 1), (ee by src, gather
    # dst, wt ee_weight), (rr by src, gather dst, wt 1).
    m_t = np.concatenate([er_dst, ee_src, rr_src])
    m_g = np.concatenate([er_src, ee_dst, rr_dst])
    m_w = np.concatenate(
        [np.ones(er_dst.shape[0], np.float32), ee_w, np.ones(rr_src.shape[0], np.float32)]
    )
    mn_t, mn_g, mn_w, mn_s, mn_e = _bucketize(m_t, m_g, m_w, N, nloc, n_windows)
    mn_cnt = _region_counts(mn_t, mn_g, mn_s, mn_e, n_windows, region_size)
    KMr = [int(np.max((mn_cnt[..., r] + P - 1) // P)) for r in range(4)]
    KM = int(sum(KMr))

    a_g16, a_t, a_v = _pack_stream_regions(
        at_t, at_g, at_w, at_s, at_e, n_windows, nloc, KAr, region_size)
    m_g16, m_t2, m_w2 = _pack_stream_regions(
        mn_t, mn_g, mn_w, mn_s, mn_e, n_windows, nloc, KMr, region_size)

    KT = KA + KM
    # meta layout per (core, window): (128 partitions, 2*KT) int32:
    #   [:, 0:KT]    t_local as f32 bits (attn tiles then mean tiles)
    #   [:, KT:2KT]  v / wt as f32 bits
    # partition p, col k corresponds to edge slot (tile k, lane p).
    def to_pk(a):  # (NC, NW, K*P) -> (NC, NW, P, K)
        return a.reshape(N_CORES, n_windows, -1, P).transpose(0, 1, 3, 2)

    meta = np.zeros((N_CORES, n_windows, P, 2 * KT), dtype=np.int32)
    meta[..., 0:KA] = to_pk(a_t).view(np.int32)
    meta[..., KA:KT] = to_pk(m_t2).view(np.int32)
    meta[..., KT : KT + KA] = to_pk(a_v).view(np.int32)
    meta[..., KT + KA : 2 * KT] = to_pk(m_w2).view(np.int32)

    # wrapped int16 gather indices: (NC, NW, P, 8*KT)
    idx16 = np.concatenate([_wrap_idx16(a_g16), _wrap_idx16(m_g16)], axis=3)

    W_attn_w = np.asarray(inputs["W_attn_w"], dtype=np.float32)  # (128, 256)
    W_attn_b = np.asarray(inputs["W_attn_b"], dtype=np.float32)  # (128,)
    w0_w = np.asarray(inputs["w0_w"], dtype=np.float32)  # (1, 128)
    w0_b = float(np.asarray(inputs["w0_b"], dtype=np.float32)[0])
    W1_w = np.asarray(inputs["W1_w"], dtype=np.float32)
    W1_b = np.asarray(inputs["W1_b"], dtype=np.float32)
    W2_w = np.asarray(inputs["W2_w"], dtype=np.float32)
    W2_b = np.asarray(inputs["W2_b"], dtype=np.float32)
    W3_w = np.asarray(inputs["W3_w"], dtype=np.float32)
    W3_b = np.asarray(inputs["W3_b"], dtype=np.float32)

    WrT = np.ascontiguousarray(W_attn_w[:, :P].T)  # (D, feat)
    WhT = np.ascontiguousarray(W_attn_w[:, P:].T)  # (D, feat)
    w0col = np.ascontiguousarray(w0_w.T)  # (feat, 1)
    ab_col = np.ascontiguousarray(W_attn_b[:, None])  # (feat, 1)
    W1T = np.ascontiguousarray(W1_w.T)
    W2T = np.ascontiguousarray(W2_w.T)
    W3T = np.ascontiguousarray(W3_w.T)
    b1r = np.ascontiguousarray(W1_b[None, :])
    b2r = np.ascontiguousarray(W2_b[None, :])
    b3r = np.ascontiguousarray(W3_b[None, :])
    ones_row = np.ones((1, P), dtype=np.float32)
    w0b_col = np.full((P, 1), w0_b, dtype=np.float32)
    iota_row = np.tile(np.arange(P, dtype=np.float32)[None, :], (P, 1))

    in_maps = []
    for c in range(N_CORES):
        sl = node_emb[c * nloc : (c + 1) * nloc]
        nodeT = np.zeros((P, nloc_pad), dtype=np.float32)
        nodeT[:, :nloc] = sl.T
        in_maps.append(
            {
                "node_emb": node_emb,
                "nodeT": nodeT,
                "meta": np.ascontiguousarray(meta[c]),
                "idx16": np.ascontiguousarray(idx16[c]),
                "WrT": WrT,
                "WhT": WhT,
                "w0col": w0col,
                "ab_col": ab_col,
                "W1T": W1T,
                "W2T": W2T,
                "W3T": W3T,
                "b1r": b1r,
                "b2r": b2r,
                "b3r": b3r,
                "ones_row": ones_row,
                "w0b_col": w0b_col,
                "iota_row": iota_row,
            }
        )

    cfg = dict(
        N=N, nloc=nloc, n_windows=n_windows, nloc_pad=nloc_pad,
        KA=KA, KM=KM, KT=KT, w0_b=w0_b,
        KAr=tuple(KAr), KMr=tuple(KMr), region_size=region_size,
    )
    return cfg, in_maps


# ----------------------------------------------------------------------------
# Bass kernel builder
# ----------------------------------------------------------------------------

def build_bass(cfg):
    N = cfg["N"]
    NW = cfg["n_windows"]
    KA, KM, KT = cfg["KA"], cfg["KM"], cfg["KT"]
    KAr, KMr = list(cfg["KAr"]), list(cfg["KMr"])
    region_size = cfg["region_size"]
    nloc_pad = cfg["nloc_pad"]
    w0_b = cfg["w0_b"]

    nc = bacc.Bacc(trn_type="TRN2", num_swdge_queues=4)

    node_emb = nc.dram_tensor("node_emb", [N, P], F32, kind="ExternalInput")
    nodeT = nc.dram_tensor("nodeT", [P, nloc_pad], F32, kind="ExternalInput")
    meta_d = nc.dram_tensor("meta", [NW, P, 2 * KT], I32, kind="ExternalInput")
    idx16_d = nc.dram_tensor("idx16", [NW, P, 8 * KT], mybir.dt.int16, kind="ExternalInput")
    WrT_d = nc.dram_tensor("WrT", [P, P], F32, kind="ExternalInput")
    WhT_d = nc.dram_tensor("WhT", [P, P], F32, kind="ExternalInput")
    w0col_d = nc.dram_tensor("w0col", [P, 1], F32, kind="ExternalInput")
    ab_col_d = nc.dram_tensor("ab_col", [P, 1], F32, kind="ExternalInput")
    W1T_d = nc.dram_tensor("W1T", [P, P], F32, kind="ExternalInput")
    W2T_d = nc.dram_tensor("W2T", [P, P], F32, kind="ExternalInput")
    W3T_d = nc.dram_tensor("W3T", [P, P], F32, kind="ExternalInput")
    b1r_d = nc.dram_tensor("b1r", [1, P], F32, kind="ExternalInput")
    b2r_d = nc.dram_tensor("b2r", [1, P], F32, kind="ExternalInput")
    b3r_d = nc.dram_tensor("b3r", [1, P], F32, kind="ExternalInput")
    ones_d = nc.dram_tensor("ones_row", [1, P], F32, kind="ExternalInput")
    w0b_d = nc.dram_tensor("w0b_col", [P, 1], F32, kind="ExternalInput")
    iota_d = nc.dram_tensor("iota_row", [P, P], F32, kind="ExternalInput")

    out_d = nc.dram_tensor("out", [nloc_pad, P], F32, kind="ExternalOutput")

    with tile.TileContext(nc) as tc:
        with (
            tc.tile_pool(name="const", bufs=1) as cpool,
            tc.tile_pool(name="win", bufs=2) as wpool,
            tc.tile_pool(name="work", bufs=3) as kpool,
            tc.tile_pool(name="psacc", bufs=1, space="PSUM") as ps_acc,
            tc.tile_pool(name="pstr", bufs=2, space="PSUM") as ps_tr,
            tc.tile_pool(name="psecol", bufs=1, space="PSUM") as ps_ecol,
        ):
            # ---- constants ----
            nc.gpsimd.load_library(mlp)
            ident = cpool.tile([P, P], F32, tag="ident")
            make_identity(nc, ident[:])
            iota_sb = cpool.tile([P, P], F32, tag="iota")
            nc.sync.dma_start(out=iota_sb[:], in_=iota_d[:, :])
            WrT_sb = cpool.tile([P, P], F32, tag="WrT")
            nc.sync.dma_start(out=WrT_sb[:], in_=WrT_d[:, :])
            WhT_sb = cpool.tile([P, P], F32, tag="WhT")
            nc.sync.dma_start(out=WhT_sb[:], in_=WhT_d[:, :])
            w0col_sb = cpool.tile([P, 1], F32, tag="w0col")
            nc.sync.dma_start(out=w0col_sb[:], in_=w0col_d[:, :])
            ab_sb = cpool.tile([P, 1], F32, tag="ab")
            nc.sync.dma_start(out=ab_sb[:], in_=ab_col_d[:, :])
            W1T_sb = cpool.tile([P, P], F32, tag="W1T")
            nc.sync.dma_start(out=W1T_sb[:], in_=W1T_d[:, :])
            W2T_sb = cpool.tile([P, P], F32, tag="W2T")
            nc.sync.dma_start(out=W2T_sb[:], in_=W2T_d[:, :])
            W3T_sb = cpool.tile([P, P], F32, tag="W3T")
            nc.sync.dma_start(out=W3T_sb[:], in_=W3T_d[:, :])
            b1_sb = cpool.tile([1, P], F32, tag="b1")
            nc.sync.dma_start(out=b1_sb[:], in_=b1r_d[:, :])
            b2_sb = cpool.tile([1, P], F32, tag="b2")
            nc.sync.dma_start(out=b2_sb[:], in_=b2r_d[:, :])
            b3_sb = cpool.tile([1, P], F32, tag="b3")
            nc.sync.dma_start(out=b3_sb[:], in_=b3r_d[:, :])
            ones_sb = cpool.tile([1, P], F32, tag="ones")
            nc.sync.dma_start(out=ones_sb[:], in_=ones_d[:, :])
            w0b_sb = cpool.tile([P, 1], F32, tag="w0b")
            nc.sync.dma_start(out=w0b_sb[:], in_=w0b_d[:, :])

            for w in range(NW):
                # ---- per-window loads ----
                meta_sb = wpool.tile([P, 2 * KT], I32, tag="meta")
                nc.sync.dma_start(out=meta_sb[:], in_=meta_d[w, :, :])
                idx_sb = wpool.tile([P, 8 * KT], mybir.dt.int16, tag="idx16")
                nc.sync.dma_start(out=idx_sb[:], in_=idx16_d[w, :, :])
                nodeT_sb = wpool.tile([P, P], F32, tag="nodeT")
                nc.sync.dma_start(out=nodeT_sb[:], in_=nodeT[:, w * P : (w + 1) * P])

                t_all = meta_sb[:, 0:KT].bitcast(F32)
                v_all = meta_sb[:, KT : 2 * KT].bitcast(F32)

                # region-pure dma_gather calls (int16 idx, 4 SWDGE queues)
                r_big = wpool.tile([P, KA * P], F32, tag="r_big")
                g_big = wpool.tile([P, KM * P], F32, tag="g_big")
                qn = 0
                for big, Ks, slot0 in ((r_big, KAr, 0), (g_big, KMr, KA)):
                    off = 0
                    for r in range(4):
                        Kr = Ks[r]
                        if Kr == 0:
                            continue
                        ni = Kr * P
                        base = r * region_size
                        hi = min(N - base, region_size)
                        nc.gpsimd.dma_gather(
                            big[:, off * P : (off + Kr) * P].rearrange(
                                "p (t e) -> p t e", e=P
                            ),
                            node_emb[base : base + hi, :],
                            idx_sb[:, (slot0 + off) * 8 : (slot0 + off + Kr) * 8],
                            ni,
                            ni,
                            P,
                            queue_num=qn % 4,
                        )
                        qn += 1
                        off += Kr

                # WH_w = window @ WhT   (nodes x feat)
                wh_ps = ps_tr.tile([P, P], F32, tag="tr")
                nc.tensor.matmul(out=wh_ps[:], lhsT=nodeT_sb[:], rhs=WhT_sb[:],
                                 start=True, stop=True)
                WH_sb = kpool.tile([P, P], F32, tag="WH")
                nc.vector.tensor_copy(out=WH_sb[:], in_=wh_ps[:])

                A_ps = ps_acc.tile([P, P + 1], F32, tag="A")
                M_ps = ps_acc.tile([P, P + 1], F32, tag="M")

                def build_P(slot0, g, tag):
                    # one-hot tiles for tiles [slot0, slot0+g): (P, g*128)
                    Pb = kpool.tile([P, g * P], F32, tag=tag)
                    nc.vector.tensor_tensor(
                        out=Pb[:].rearrange("p (k e) -> p k e", e=P),
                        in0=t_all[:, slot0 : slot0 + g]
                        .rearrange("p (k o) -> p k o", o=1)
                        .to_broadcast([P, g, P]),
                        in1=iota_sb[:]
                        .rearrange("p (o e) -> p o e", o=1)
                        .to_broadcast([P, g, P]),
                        op=mybir.AluOpType.is_equal,
                    )
                    return Pb

                # ---- attention tiles (groups of up to 4) ----
                ecol_ps = ps_ecol.tile([P, KA], F32, tag="ecol")
                tanhT_sb = kpool.tile([P, KA * P], F32, tag="tanhT")
                P_att = []
                for g0 in range(0, KA, 4):
                    g = min(4, KA - g0)
                    Pb = build_P(g0, g, "Pa")
                    P_att.append((g0, g, Pb))
                    # transposes into shared PSUM banks
                    rT_ps = ps_tr.tile([P, g * P], F32, tag="tr")
                    for k in range(g):
                        nc.tensor.transpose(
                            out=rT_ps[:, k * P : (k + 1) * P],
                            in_=r_big[:, (g0 + k) * P : (g0 + k + 1) * P],
                            identity=ident[:],
                        )
                    rT_sb = kpool.tile([P, g * P], F32, tag="rT")
                    nc.vector.tensor_copy(out=rT_sb[:], in_=rT_ps[:])
                    PT_ps = ps_tr.tile([P, g * P], F32, tag="tr")
                    for k in range(g):
                        nc.tensor.transpose(
                            out=PT_ps[:, k * P : (k + 1) * P],
                            in_=Pb[:, k * P : (k + 1) * P],
                            identity=ident[:],
                        )
                    PT_sb = kpool.tile([P, g * P], F32, tag="PT")
                    nc.scalar.copy(out=PT_sb[:], in_=PT_ps[:])

                    # e_preT = WrT.T @ rT + WH.T @ PT   (feat x g*128)
                    eT_ps = ps_tr.tile([P, g * P], F32, tag="epre")
                    nc.tensor.matmul(out=eT_ps[:], lhsT=WrT_sb[:], rhs=rT_sb[:],
                                     start=True, stop=False)
                    nc.tensor.matmul(out=eT_ps[:], lhsT=WH_sb[:], rhs=PT_sb[:],
                                     start=False, stop=True)
                    nc.scalar.activation(
                        out=tanhT_sb[:, g0 * P : (g0 + g) * P], in_=eT_ps[:],
                        func=mybir.ActivationFunctionType.Tanh, bias=ab_sb[:],
                    )
                    for k in range(g):
                        nc.tensor.matmul(
                            out=ecol_ps[:, g0 + k : g0 + k + 1],
                            lhsT=tanhT_sb[:, (g0 + k) * P : (g0 + k + 1) * P],
                            rhs=w0col_sb[:],
                            start=True, stop=True, skip_group_check=True,
                        )
                # w = exp(e + b0) * v   (one exp per window)
                wraw_sb = kpool.tile([P, KA], F32, tag="wraw")
                nc.scalar.activation(
                    out=wraw_sb[:], in_=ecol_ps[:],
                    func=mybir.ActivationFunctionType.Exp, bias=w0b_sb[:],
                )
                wcol_sb = kpool.tile([P, KA], F32, tag="wcol")
                nc.vector.tensor_tensor(
                    out=wcol_sb[:], in0=wraw_sb[:], in1=v_all[:, 0:KA],
                    op=mybir.AluOpType.mult,
                )
                # messages + scatter
                for g0, g, Pb in P_att:
                    msg_sb = kpool.tile([P, g * (P + 1)], F32, tag="msga")
                    for k in range(g):
                        nc.vector.tensor_scalar_mul(
                            msg_sb[:, k * (P + 1) : k * (P + 1) + P],
                            r_big[:, (g0 + k) * P : (g0 + k + 1) * P],
                            wcol_sb[:, g0 + k : g0 + k + 1],
                        )
                    nc.vector.tensor_copy(
                        out=msg_sb[:].rearrange("p (k e) -> p k e", e=P + 1)[:, :, P : P + 1],
                        in_=wcol_sb[:, g0 : g0 + g].rearrange("p (k o) -> p k o", o=1),
                    )
                    for k in range(g):
                        kk = g0 + k
                        nc.tensor.matmul(
                            out=A_ps[:],
                            lhsT=Pb[:, k * P : (k + 1) * P],
                            rhs=msg_sb[:, k * (P + 1) : (k + 1) * (P + 1)],
                            start=(kk == 0), stop=(kk == KA - 1),
                            skip_group_check=True,
                        )

                # ---- mean tiles (groups of up to 4) ----
                for g0 in range(0, KM, 4):
                    g = min(4, KM - g0)
                    Pb = build_P(KA + g0, g, "Pm")
                    msg_sb = kpool.tile([P, g * (P + 1)], F32, tag="msgm")
                    for k in range(g):
                        nc.scalar.activation(
                            out=msg_sb[:, k * (P + 1) : k * (P + 1) + P],
                            in_=g_big[:, (g0 + k) * P : (g0 + k + 1) * P],
                            func=mybir.ActivationFunctionType.Copy,
                            scale=v_all[:, KA + g0 + k : KA + g0 + k + 1],
                        )
                    nc.vector.tensor_copy(
                        out=msg_sb[:].rearrange("p (k e) -> p k e", e=P + 1)[:, :, P : P + 1],
                        in_=v_all[:, KA + g0 : KA + g0 + g].rearrange("p (k o) -> p k o", o=1),
                    )
                    for k in range(g):
                        kk = g0 + k
                        nc.tensor.matmul(
                            out=M_ps[:],
                            lhsT=Pb[:, k * P : (k + 1) * P],
                            rhs=msg_sb[:, k * (P + 1) : (k + 1) * (P + 1)],
                            start=(kk == 0), stop=(kk == KM - 1),
                            skip_group_check=True,
                        )

                # ---- normalize ----
                sden_sb = kpool.tile([P, 1], F32, tag="sden")
                nc.vector.tensor_scalar_add(sden_sb[:], A_ps[:, P : P + 1], 1e-9)
                srec_sb = kpool.tile([P, 1], F32, tag="srec")
                nc.vector.reciprocal(srec_sb[:], sden_sb[:])
                attn_sb = kpool.tile([P, P], F32, tag="attn")
                nc.scalar.activation(
                    out=attn_sb[:], in_=A_ps[:, 0:P],
                    func=mybir.ActivationFunctionType.Copy, scale=srec_sb[:],
                )
                cden_sb = kpool.tile([P, 1], F32, tag="cden")
                nc.vector.tensor_scalar_max(cden_sb[:], M_ps[:, P : P + 1], 1.0)
                crec_sb = kpool.tile([P, 1], F32, tag="crec")
                nc.vector.reciprocal(crec_sb[:], cden_sb[:])
                mean_sb = kpool.tile([P, P], F32, tag="mean")
                nc.scalar.activation(
                    out=mean_sb[:], in_=M_ps[:, 0:P],
                    func=mybir.ActivationFunctionType.Copy, scale=crec_sb[:],
                )

                # ---- final matmuls ----
                aT_ps = ps_tr.tile([P, P], F32, tag="tr")
                nc.tensor.transpose(out=aT_ps[:], in_=attn_sb[:], identity=ident[:])
                aT_sb = kpool.tile([P, P], F32, tag="aT")
                nc.vector.tensor_copy(out=aT_sb[:], in_=aT_ps[:])
                mT_ps = ps_tr.tile([P, P], F32, tag="tr")
                nc.tensor.transpose(out=mT_ps[:], in_=mean_sb[:], identity=ident[:])
                mT_sb = kpool.tile([P, P], F32, tag="mT")
                nc.vector.tensor_copy(out=mT_sb[:], in_=mT_ps[:])

                out_sb = kpool.tile([P, P], F32, tag="out")
                acc_sb = kpool.tile([P, P], F32, tag="acc")
                for term, (lhsT_sb, Wt_sb, b_sb) in enumerate(
                    [
                        (nodeT_sb, W1T_sb, b1_sb),
                        (aT_sb, W2T_sb, b2_sb),
                        (mT_sb, W3T_sb, b3_sb),
                    ]
                ):
                    O_ps = ps_tr.tile([P, P], F32, tag="tr")
                    nc.tensor.matmul(out=O_ps[:], lhsT=ones_sb[:], rhs=b_sb[:],
                                     start=True, stop=False, skip_group_check=True)
                    nc.tensor.matmul(out=O_ps[:], lhsT=lhsT_sb[:], rhs=Wt_sb[:],
                                     start=False, stop=True, skip_group_check=True)
                    if term == 0:
                        nc.scalar.activation(
                            out=acc_sb[:], in_=O_ps[:],
                            func=mybir.ActivationFunctionType.Tanh,
                        )
                    else:
                        t_sb = kpool.tile([P, P], F32, tag="tterm")
                        nc.scalar.activation(
                            out=t_sb[:], in_=O_ps[:],
                            func=mybir.ActivationFunctionType.Tanh,
                        )
                        nc.vector.tensor_add(
                            out=(out_sb[:] if term == 2 else acc_sb[:]),
                            in0=acc_sb[:], in1=t_sb[:],
                        )
                nc.sync.dma_start(out=out_d[w * P : (w + 1) * P, :], in_=out_sb[:])

    nc.compile()
    return nc


# ----------------------------------------------------------------------------
# Entry point
# ----------------------------------------------------------------------------

_CACHE = {}


def _get_nc(cfg):
    key = tuple(sorted((k, v) for k, v in cfg.items()))
    if key not in _CACHE:
        _CACHE[key] = build_bass(cfg)
    return _CACHE[key]


def kernel(**inputs):
    cfg, in_maps = prepare_inputs(inputs)
    nc = _get_nc(cfg)
    res = bass_utils.run_bass_kernel_spmd(nc, in_maps, core_ids=list(range(N_CORES)))
    nloc = cfg["nloc"]
    outs = [r["out"][:nloc] for r in res.results]
    return np.ascontiguousarray(np.concatenate(outs, axis=0), dtype=np.float32)
